# revision 34
# baseline (speedup 1.0000x reference)
"""Trainium2 Bass kernel for nn_Attention_57432302682539.

Reference computation (B=32, S=4096, D=256, H=256):
    inp = x @ W_in.T + b_in                                  # [B, H]
    branch_i: ctx = einsum('bsd,hd->bhs', context, Wc_i) + bc_i
              att_i = einsum('h,bhs->bs', V_i, tanh(inp[:,:,None] + ctx))
    att = concat(att_0..3, axis=1)                           # [B, 4S]
    att = 10*tanh(att)  (mask is all ones -> where() is identity)
    out = softmax(att, axis=0)                               # over batch

Sharding: S is split 8 ways (512 positions per core); every core holds all
32 batches, so the dim-0 (batch) softmax is entirely core-local and no
collective is needed.

Per-core pipeline (ACT-bound: 256 tanh instrs x ~(512+222)cyc @1.2GHz):
  - context DMA'd in 4-batch blocks with one-block-ahead prefetch; groups
    iterate (block, br) so a group's 4 batches share per-h-tile SBUF tiles
    [128, 4, 512]
  - main matmuls: WcT.T @ ctxT -> PSUM [128h, 512s], K=256 via 2 accum steps
  - ACT tanh with per-partition bias (inp[b]+bc) -> fp16 tile slice
  - V-weighting: 3 wide DVE ops per group (FD=2048; 4x tensor_scalar muls +
    2x tensor_tensor add) instead of 12 narrow ones; last group uses narrow
    ops so the tail drain interleaves with its V-sum matmuls
  - V-dot over h: matmul with a sliding-window ones buffer whose single
    nonzero column routes each (b,br) partition-sum into row br*32+b of one
    PSUM accumulator [128, 512]
  - exp(10*tanh(att)) on ACT in fp16 (keeps the den matmul at 1 cyc/row and
    halves the output DMA); denominator via a [128,128] group-select matmul
    that replicates the 4 branch-group sums to all partitions (no broadcast
    DMA); DVE reciprocal + multiply
  - output DMA issued from the SWDGE/Pool queue so the SP queue (context
    loads) never blocks on the softmax tail at the loop boundary -> the
    next iteration's context prefetch overlaps the tail.
"""

import os
import numpy as np

B, S, D, H = 32, 4096, 256, 256
NCORES = 8
SC = S // NCORES          # 512 s-positions per core
NBR = 4                   # branches
HT = 2                    # h tiles of 128
DTILES = 2                # d tiles of 128
P = 128

_CACHE = {}


def _build_nc(dt_name: str, repeat: int = 1, loop_n: int = 0, variant: str = "full"):
    """Build the Bass module. dt_name in ('float16', 'bfloat16').

    repeat>1 unrolls the whole computation N times inside the NEFF; loop_n>0
    instead wraps it in a hardware For_i loop with that trip count (both for
    on-device timing via wall-clock differencing); the result is unchanged.
    """
    import concourse.bass as bass
    import concourse.mybir as mybir
    import concourse.tile as tile
    from concourse import bacc

    DT = getattr(mybir.dt, dt_name)
    F32 = mybir.dt.float32
    AF = mybir.ActivationFunctionType

    nc = bacc.Bacc(
        trn_type="TRN2",
        use_seq_codegen=os.environ.get("KERNEL_SEQ_CODEGEN", "0") == "1",
    )

    # Per-core external inputs (host-preprocessed).
    ctxT = nc.dram_tensor("ctxT", [B, DTILES, P, SC], DT, kind="ExternalInput")
    wcT = nc.dram_tensor("wcT", [P, DTILES, NBR, HT, P], DT, kind="ExternalInput")
    ones_w = nc.dram_tensor("ones_w", [P, 256], DT, kind="ExternalInput")
    vcols = nc.dram_tensor("vcols", [P, NBR, HT], F32, kind="ExternalInput")
    winT = nc.dram_tensor("winT", [P, DTILES, HT, P], DT, kind="ExternalInput")
    xT = nc.dram_tensor("xT", [P, DTILES, B], DT, kind="ExternalInput")
    bcomb = nc.dram_tensor("bcomb", [P, NBR, HT], F32, kind="ExternalInput")
    sel = nc.dram_tensor("sel", [P, P], DT, kind="ExternalInput")
    out = nc.dram_tensor("out", [B, NBR, SC], DT, kind="ExternalOutput")

    from contextlib import ExitStack

    # tanh output to PSUM would shave ACT's per-instr SBUF-write bubble
    # (222 -> 172 cycles), but walrus codegen rejects 16-bit ACTIVATE
    # writes into PSUM tile slices (tensor3d_valid), so this stays off.
    TANH_PSUM = os.environ.get("KERNEL_TANH_PSUM", "0") == "1"

    with tile.TileContext(nc) as tc, ExitStack() as ctx:
        const = ctx.enter_context(tc.tile_pool(name="const", bufs=1))
        ctxp = ctx.enter_context(tc.tile_pool(name="ctxp", bufs=3))
        finalp = ctx.enter_context(tc.tile_pool(name="finalp", bufs=2))
        if TANH_PSUM:
            tanhp = ctx.enter_context(
                tc.tile_pool(name="tanhp", bufs=4, space="PSUM"))
            ps_main = ctx.enter_context(
                tc.tile_pool(name="ps_main", bufs=3, space="PSUM"))
        else:
            tanhp = ctx.enter_context(tc.tile_pool(name="tanhp", bufs=4))
            ps_main = ctx.enter_context(
                tc.tile_pool(name="ps_main", bufs=6, space="PSUM"))
        ps_att = ctx.enter_context(tc.tile_pool(name="ps_att", bufs=1, space="PSUM"))
        wsump = ctx.enter_context(tc.tile_pool(name="wsump", bufs=6))

        # ---- constants into SBUF ----
        # wcT first: the head-of-pipeline matmuls need it (with ctx b0, which
        # is issued right behind it inside emit_body on the same queue)
        wcT_sb = const.tile([P, DTILES, NBR, HT, P], DT)
        nc.sync.dma_start(out=wcT_sb, in_=wcT[:])
        winT_sb = const.tile([P, DTILES, HT, P], DT)
        nc.sync.dma_start(out=winT_sb, in_=winT[:])
        xT_sb = const.tile([P, DTILES, B], DT)
        nc.sync.dma_start(out=xT_sb, in_=xT[:])
        bcomb_sb = const.tile([P, NBR, HT], F32)
        nc.sync.dma_start(out=bcomb_sb, in_=bcomb[:])
        ones_sb = const.tile([P, 256], DT)
        nc.sync.dma_start(out=ones_sb, in_=ones_w[:])
        vcols_sb = const.tile([P, NBR, HT], F32)
        nc.sync.dma_start(out=vcols_sb, in_=vcols[:])
        sel_sb = const.tile([P, P], DT)
        nc.sync.dma_start(out=sel_sb, in_=sel[:])

        # prime the ACT spline tables (tanh+exp share one set) while the
        # first context DMA is in flight, instead of stalling the first tile
        prime = const.tile([1, 1], F32)
        nc.gpsimd.memset(prime[:], 0.0)
        prime2 = const.tile([1, 1], F32)
        nc.scalar.activation(prime2[:], prime[:], AF.Tanh)

        # ---- inp = x @ W_in.T (+ b_in + bc), laid h-on-partitions ----
        # bias_all[:, br, ht, b] = sum_d W_in[h,d] x[b,d] + b_in[h] + bc_br[h]
        bias_all = const.tile([P, NBR, HT, B], F32)
        for ht in range(HT):
            ps_inp = ps_main.tile([P, B], F32, tag="ps_ctx", name="ps_inp")
            for dti in range(DTILES):
                nc.tensor.matmul(
                    ps_inp[:],
                    lhsT=winT_sb[:, dti, ht],
                    rhs=xT_sb[:, dti],
                    start=(dti == 0),
                    stop=(dti == DTILES - 1),
                )
            for br in range(NBR):
                nc.vector.tensor_scalar_add(
                    bias_all[:, br, ht], ps_inp[:], bcomb_sb[:, br, ht : ht + 1]
                )

        # ---- main pipeline over 4-batch blocks x branches ----
        # Within a block of 4 batches, iterate br; the 4 (b, br) tanh tiles
        # of a (block, br) group live in one mega-tile so the V-weighting
        # runs as 3 wide DVE ops (FD=2048) instead of 12 narrow ones.
        BBLK = int(os.environ.get("KERNEL_BBLK", "2" if TANH_PSUM else "4"))
        groups = [(b0, br) for b0 in range(0, B, BBLK) for br in range(NBR)]
        ngroups = len(groups)
        GLAG = int(os.environ.get("KERNEL_GLAG", "1"))

        def emit_body():
            att_ps = ps_att.tile([P, SC], F32, tag="att", name="att_ps")
            pend_dve = []   # (blk_tile_or_list, b0, br)
            pend_mm = []    # (s_blk, i, b, br)
            dve_idx = 0
            mm_idx = 0
            nvsum = [0]
            skipv = variant in ("no_vdot", "mm_only", "mm_half", "dma_only")

            def emit_dve(ig, narrow=False):
                # narrow=True: per-batch ops so the drain interleaves with
                # the V-sum matmuls (used for the last group = kernel tail)
                Tblk, b0, br = pend_dve[ig]
                u_t = wsump.tile([P, BBLK, SC], DT, tag="acc", name="u_t")
                w_t = wsump.tile([P, BBLK, SC], DT, tag="acc", name="w_t")
                s_t = wsump.tile([P, BBLK, SC], DT, tag="acc", name="s_t")
                splits = list(range(BBLK)) if narrow else [slice(0, BBLK)]
                for sl in splits:
                    nc.vector.tensor_scalar_mul(
                        u_t[:, sl], Tblk[0][:, sl], vcols_sb[:, br, 0:1])
                    nc.vector.tensor_scalar_mul(
                        w_t[:, sl], Tblk[1][:, sl], vcols_sb[:, br, 1:2])
                    nc.vector.tensor_add(s_t[:, sl], u_t[:, sl], w_t[:, sl])
                    for i in ([sl] if narrow else list(range(BBLK))):
                        pend_mm.append((s_t, i, b0 + i, br))

            def emit_vsum(ip):
                s_t, i, b, br = pend_mm[ip]
                j = br * 32 + b
                nvsum[0] += 1
                nc.tensor.matmul(
                    att_ps[:],
                    lhsT=ones_sb[:, 127 - j : 255 - j],
                    rhs=s_t[:, i],
                    start=(ip == 0),
                    stop=(nvsum[0] == (1 if skipv else B * NBR)),
                )

            def load_blk(b0):
                # one DMA for batches b0..b0+3, both d-tiles:
                # dst[p, bb, dt, s] <- ctxT[b0+bb, dt, p, s]
                tp = ctxp.tile([P, BBLK, DTILES, SC], DT, tag="ctx", name="ctxblk")
                src = ctxT[b0 : b0 + BBLK].rearrange("b dt p s -> p b dt s")
                nc.sync.dma_start(out=tp, in_=src)
                return tp

            # first context block: batch 0 gets its own transfer so the
            # first matmul doesn't wait for the whole block; block 1 is
            # preloaded too, and each block k prefetches k+1 one full block
            # (4 branches of work) ahead
            tp0 = ctxp.tile([P, BBLK, DTILES, SC], DT, tag="ctx", name="ctxblk")
            nc.sync.dma_start(
                out=tp0[:, 0], in_=ctxT[0].rearrange("dt p s -> p dt s")
            )
            nc.sync.dma_start(
                out=tp0[:, 1:], in_=ctxT[1:BBLK].rearrange("b dt p s -> p b dt s")
            )
            cblk = {0: tp0}
            if variant != "no_dma":
                cblk[BBLK] = load_blk(BBLK)

            for ig, (b0, br) in enumerate(groups):
                if br == 0 and 0 < b0 <= B - 2 * BBLK and variant != "no_dma":
                    cblk[b0 + BBLK] = load_blk(b0 + BBLK)
                    cblk.pop(b0 - BBLK, None)
                blk = cblk[b0] if variant != "no_dma" else cblk[0]
                ttp = None
                if variant not in ("no_act", "mm_only", "mm_half", "dma_only"):
                    # one [P, BBLK, SC] tile per h-tile (1 PSUM bank each
                    # when TANH_PSUM and BBLK=2)
                    ttp = [
                        tanhp.tile([P, BBLK, SC], DT, tag="tanh", name="ttp")
                        for _ in range(HT)
                    ]
                for i in range(BBLK):
                    b = b0 + i
                    bb = 0 if variant == "no_dma" else i
                    for ht in range(HT):
                        if variant not in ("no_mm", "dma_only"):
                            ps = ps_main.tile([P, SC], F32, name="ps_ctx")
                            ndt = 1 if variant == "mm_half" else DTILES
                            for dti in range(ndt):
                                nc.tensor.matmul(
                                    ps[:],
                                    lhsT=wcT_sb[:, dti, br, ht],
                                    rhs=blk[:, bb, dti][:],
                                    start=(dti == 0),
                                    stop=(dti == ndt - 1),
                                )
                        if ttp is not None:
                            src_ap = blk[:, bb, 0][:] if variant == "no_mm" else ps[:]
                            nc.scalar.activation(
                                ttp[ht][:, i], src_ap, AF.Tanh,
                                bias=bias_all[:, br, ht, b : b + 1],
                            )
                if ttp is not None:
                    pend_dve.append(((ttp[0], ttp[1]), b0, br))
                else:
                    pend_dve.append(((blk[:, :, 0], blk[:, :, 1]), b0, br))
                if skipv:
                    if ig == 0:
                        emit_dve(0)
                        emit_vsum(0)
                        dve_idx = 1
                        mm_idx = 1
                    continue
                if ig >= GLAG:
                    emit_dve(dve_idx)
                    dve_idx += 1
                while mm_idx < len(pend_mm) - BBLK:
                    emit_vsum(mm_idx)
                    mm_idx += 1
            if not skipv:
                while dve_idx < ngroups:
                    emit_dve(dve_idx, narrow=(dve_idx == ngroups - 1))
                    dve_idx += 1
                    while mm_idx < len(pend_mm):
                        emit_vsum(mm_idx)
                        mm_idx += 1

            # ---- softmax over batch (local: all 32 batches on this core) ----
            # att rows are laid p = br*32 + b
            th = finalp.tile([P, SC], F32, tag="th", name="th")
            nc.scalar.activation(th[:], att_ps[:], AF.Tanh)
            # fp16 exp keeps the den matmul at 1 cyc/row (fp32 would be 4x)
            # and halves the output DMA; e^10 = 22k fits fp16 range
            ex = finalp.tile([P, SC], DT, tag="ex", name="ex")
            nc.scalar.activation(ex[:], th[:], AF.Exp, scale=10.0)

            # den replicated to all 128 partitions: sel[p, m] = (p//32 == m//32)
            # -> den_ps[m, s] = group-sum of ex for m's group; no broadcast DMA
            den_ps = ps_main.tile([P, SC], F32, tag="ps_ctx", name="den_ps")
            nc.tensor.matmul(
                den_ps[:], lhsT=sel_sb[:], rhs=ex[:], start=True, stop=True
            )

            invrep = finalp.tile([P, SC], F32, tag="invrep", name="invrep")
            nc.vector.reciprocal(invrep[:], den_ps[:])

            outv = finalp.tile([P, SC], DT, tag="outv", name="outv")
            nc.vector.tensor_mul(outv[:], ex[:], invrep[:])

            # one DMA: src partitions p = br*32+b -> dst out[b, br, :].
            # Issued from the SWDGE (Pool) queue so the SP queue — which owns
            # the context loads — never blocks on the softmax tail at the
            # loop boundary; SP prefetches the next iteration's ctx instead.
            o = out[:]
            dst = bass.AP(
                tensor=o.tensor, offset=o.offset,
                ap=[[SC, NBR], [NBR * SC, B], [1, SC]],
            )
            nc.gpsimd.dma_start(out=dst, in_=outv[:])

        if loop_n:
            import concourse.mybir as _mb

            with tc.For_i(
                0,
                loop_n,
                1,
                hint_engines=(
                    _mb.EngineType.PE,
                    _mb.EngineType.Activation,
                    _mb.EngineType.DVE,
                    _mb.EngineType.SP,
                    _mb.EngineType.Pool,
                ),
                staggered_reset=True,
            ):
                emit_body()
        else:
            for _rep in range(repeat):
                emit_body()

    nc.compile()
    return nc


def _host_prep(inputs, np_dt):
    """Build the per-core input maps from the full problem inputs."""
    x = np.asarray(inputs["x"], np.float32)
    context = np.ascontiguousarray(np.asarray(inputs["context"], np.float32))
    W_in = np.asarray(inputs["W_in"], np.float32)
    b_in = np.asarray(inputs["b_in"], np.float32)
    Wc = np.stack(
        [np.asarray(inputs[f"Wc{i}"], np.float32) for i in range(NBR)]
    )  # [br, h, d]
    bc = np.stack([np.asarray(inputs[f"bc{i}"], np.float32) for i in range(NBR)])
    V = np.stack([np.asarray(inputs[f"V{i}"], np.float32) for i in range(NBR)])

    # wcT[p, dt, br, ht, j] = Wc[br, ht*128+j, dt*128+p]
    wcT = np.ascontiguousarray(
        Wc.reshape(NBR, HT, P, DTILES, P).transpose(4, 3, 0, 1, 2)
    ).astype(np_dt)

    # ones window: col 127 all-ones; slice [127-j : 255-j] puts the ones
    # column at local position j (routes partition-sums to output row j)
    ones_w = np.zeros((P, 256), np_dt)
    ones_w[:, 127] = 1.0

    # vcols[p, br, ht] = V[br, ht*128+p] (per-partition DVE scalars)
    vcols = np.ascontiguousarray(
        V.reshape(NBR, HT, P).transpose(2, 0, 1)
    ).astype(np.float32)

    # winT[p, dt, ht, j] = W_in[ht*128+j, dt*128+p]
    winT = np.ascontiguousarray(
        W_in.reshape(HT, P, DTILES, P).transpose(3, 2, 0, 1)
    ).astype(np_dt)

    # xT[p, dt, b] = x[b, dt*128+p]
    xT = np.ascontiguousarray(x.reshape(B, DTILES, P).transpose(2, 1, 0)).astype(np_dt)

    # bcomb[p, br, ht] = b_in[ht*128+p] + bc[br, ht*128+p]
    bsum = b_in[None, :] + bc  # [br, H]
    bcomb = np.ascontiguousarray(
        bsum.reshape(NBR, HT, P).transpose(2, 0, 1)
    ).astype(np.float32)

    # sel[p, m] = 1 if p//32 == m//32 (den replicated across partitions)
    sel = np.zeros((P, P), np_dt)
    for m in range(NBR):
        sel[m * 32 : (m + 1) * 32, m * 32 : (m + 1) * 32] = 1.0

    shared = dict(
        wcT=wcT, ones_w=ones_w, vcols=vcols, winT=winT, xT=xT, bcomb=bcomb, sel=sel
    )

    in_maps = []
    for k in range(NCORES):
        sl = context[:, k * SC : (k + 1) * SC, :]  # [B, SC, D]
        ctxT = np.ascontiguousarray(sl.transpose(0, 2, 1)).astype(np_dt)  # [B, D, SC]
        m = dict(shared)
        m["ctxT"] = ctxT.reshape(B, DTILES, P, SC)
        in_maps.append(m)
    return in_maps


def kernel(**inputs) -> np.ndarray:
    dt_name = os.environ.get("KERNEL_DT", "float16")
    np_dt = {"float16": np.float16, "bfloat16": None}[dt_name]
    if np_dt is None:
        import ml_dtypes

        np_dt = ml_dtypes.bfloat16

    from concourse import bass_utils

    if dt_name not in _CACHE:
        _CACHE[dt_name] = _build_nc(dt_name)
    nc = _CACHE[dt_name]

    in_maps = _host_prep(inputs, np_dt)
    res = bass_utils.run_bass_kernel_spmd(nc, in_maps, core_ids=list(range(NCORES)))

    full = np.empty((B, NBR, NCORES, SC), np.float32)
    for k in range(NCORES):
        full[:, :, k, :] = res.results[k]["out"]
    return full.reshape(B, NBR * S).astype(np.float32)


if __name__ == "__main__":
    # smoke test with random inputs
    rng = np.random.default_rng(0)
    inputs = dict(
        x=rng.standard_normal((B, H), dtype=np.float32),
        context=rng.standard_normal((B, S, D), dtype=np.float32),
        mask=np.ones((B, S), bool),
        W_in=rng.uniform(-1 / 16, 1 / 16, (H, H)).astype(np.float32),
        b_in=rng.uniform(-1 / 16, 1 / 16, (H,)).astype(np.float32),
    )
    for i in range(4):
        inputs[f"Wc{i}"] = rng.uniform(-1 / 16, 1 / 16, (H, D)).astype(np.float32)
        inputs[f"bc{i}"] = rng.uniform(-1 / 16, 1 / 16, (H,)).astype(np.float32)
        inputs[f"V{i}"] = rng.uniform(-1, 1, (H,)).astype(np.float32)
    out = kernel(**inputs)
    print("out", out.shape, out.dtype, out.sum())

